# revision 6
# baseline (speedup 1.0000x reference)
"""Causal self-attention (B=4, T=2048, C=1024, H=16, D=64) on 8 TRN2 cores.

Sharding: core c handles batch b = c//2 and head-group g = c%2 (8 heads).
Each core computes qkv projection for its heads, causal flash attention,
and a partial out-projection (row-parallel over its heads' slice of w_out).
Host sums the two partials per batch and adds biases that commute
(b_k drops under softmax; b_v/b_out fold into a host-side constant row).

Device layout notes:
- x is passed pre-transposed (xT [C, T]) twice: fp8e4 for the Q/K
  projections (fp8 DoubleRow matmuls, K=256/instruction, ~2x bf16) and
  bf16 for the V projection (fp8 anywhere in the P/v/y path blows the
  2e-2 error budget; fp8 q/k only perturbs scores pre-softmax, which
  measures ~1.2e-2). The qkv weights for q/k are pre-scaled by 16 into
  fp8's normal range; S picks up 256x, absorbed into the exp scale, and
  q's bias is pre-multiplied by 16 host-side.
- Q^T/K^T are bf16 in [j, t] layout (head A d in partitions 0:64, head B
  in 64:128) -- exactly the lhsT/rhs layout the attention matmuls need.
- S^T strips [tk=128, tq=512] per head-pair via row-tiled (tile_position)
  K=64 matmuls (the two half-height matmuls stream concurrently), exp'd
  on ACT in one instruction per strip (no max-subtraction: S/8 < ~3),
  causal boundary zeroed in-place by GPSIMD affine_select (condition is
  j' >= p for every diagonal strip). Within each (qc, pair) the boundary
  strips are processed FIRST so their cross-engine masking latency hides
  behind the full strips' exp+PV work instead of stalling the pair's
  rowsum/normalize tail.
- P@V col-tiled into one [128, 512] psum per head (A -> partitions 0:64 +
  L_A at 64 via a ones column; B -> 64:128 + L_B at 32), rowsum broadcast
  via K=1 selector matmuls, reciprocal on DVE, and the psum->sbuf y
  eviction fused with the 1/L normalize multiply.
- Emission order pipelines load -> projection -> attention: qk-proj(p0),
  V(tt0:4), attn(qc0,p0), qk-proj(p1), attn(qc0,p1), ... so ACT starts
  exp'ing ~10us in instead of after all projections.
"""

import tempfile
from contextlib import ExitStack

import numpy as np
import ml_dtypes

import concourse.bass as bass
import concourse.tile as tile
from concourse import bacc, mybir
from concourse.bass_utils import run_bass_kernel_spmd

BF16 = mybir.dt.bfloat16
F32 = mybir.dt.float32
FP8 = mybir.dt.float8e4
AF = mybir.ActivationFunctionType
ALU = mybir.AluOpType
DR = mybir.MatmulPerfMode.DoubleRow

B, T, C, H, D = 4, 2048, 1024, 16, 64
HPG = 8                 # heads per core (group)
PAIRS = HPG // 2        # head pairs per core
GW = HPG * D            # 512: group width of q/k/v
CC = C // 128           # 8 contraction chunks
TT = T // 128           # 16 t-tiles
QC = T // 512           # 4 query chunks of 512
WSCALE = 16.0           # host-side q/k weight scale (fp8 subnormal dodge)
SCALE = 1.0 / (np.sqrt(D) * WSCALE * WSCALE)   # exp scale: S_raw -> S/8
N_CORES = 8


def build_kernel():
    nc = bacc.Bacc("TRN2", target_bir_lowering=False, debug=False,
                   num_devices=N_CORES)
    xT = nc.dram_tensor("xT", [C, T], FP8, kind="ExternalInput").ap()
    xTb = nc.dram_tensor("xTb", [C, T], BF16, kind="ExternalInput").ap()
    wq = nc.dram_tensor("wq", [C, GW], FP8, kind="ExternalInput").ap()
    wk = nc.dram_tensor("wk", [C, GW], FP8, kind="ExternalInput").ap()
    wv = nc.dram_tensor("wv", [C, GW], BF16, kind="ExternalInput").ap()
    bq = nc.dram_tensor("bq", [GW], F32, kind="ExternalInput").ap()
    wo = nc.dram_tensor("wo", [GW, C], BF16, kind="ExternalInput").ap()
    out = nc.dram_tensor("out", [T, C], F32, kind="ExternalOutput").ap()

    with tile.TileContext(nc) as tc, ExitStack() as ctx:
        const_p = ctx.enter_context(tc.tile_pool(name="const", bufs=1))
        w_p = ctx.enter_context(tc.tile_pool(name="w", bufs=1))
        x_p = ctx.enter_context(tc.tile_pool(name="x", bufs=1))
        qk_p = ctx.enter_context(tc.tile_pool(name="qk", bufs=1))
        v_p = ctx.enter_context(tc.tile_pool(name="v", bufs=1))
        y_p = ctx.enter_context(tc.tile_pool(name="y", bufs=1))
        exp_p = ctx.enter_context(tc.tile_pool(name="expt", bufs=8))
        stage_p = ctx.enter_context(tc.tile_pool(name="stage", bufs=3))
        # PSUM budget (8 banks): s_ps 2x[128,1024]=4, small_ps 4x[128,512]=4
        s_ps = ctx.enter_context(tc.tile_pool(name="s_ps", bufs=2, space="PSUM"))
        small_ps = ctx.enter_context(
            tc.tile_pool(name="small_ps", bufs=4, space="PSUM"))
        rec_p = ctx.enter_context(tc.tile_pool(name="rec", bufs=4))
        bc_pool = ctx.enter_context(tc.tile_pool(name="bc_sb", bufs=4))

        # ---- constants ----
        # selector for broadcasting rowsums via PE: row64 (head A's L)
        # -> out partitions 0:64, row32 (head B's L) -> 64:128.
        sel_sb = const_p.tile([65, 128], BF16)
        nc.gpsimd.memset(sel_sb[:], 0.0)
        nc.gpsimd.memset(sel_sb[64:65, 0:64], 1.0)
        nc.gpsimd.memset(sel_sb[32:33, 64:128], 1.0)

        # ---- loads, in dependency-criticality order ----
        # sync queue: bq, then per-cc {wq, wk, xT8} (Q/K proj inputs).
        # vector queue: xTb t-chunk 0 + wv (V proj for tt 0:4), then the
        # rest of xTb, then wo (needed first at outproj(qc0)).
        bq_sb = w_p.tile([128, PAIRS], F32)
        nc.sync.dma_start(bq_sb[:], bq.rearrange("(p c) -> c p", c=128))
        wq_sb = w_p.tile([128, CC, GW], FP8)
        wk_sb = w_p.tile([128, CC, GW], FP8)
        wv_sb = w_p.tile([128, CC, GW], BF16)
        xT_sb = x_p.tile([128, CC, T], FP8)
        xTb_sb = x_p.tile([128, CC, T], BF16)
        for cc in range(CC):
            csl = slice(cc * 128, (cc + 1) * 128)
            nc.sync.dma_start(wq_sb[:, cc, :], wq[csl, :])
            nc.sync.dma_start(wk_sb[:, cc, :], wk[csl, :])
            nc.sync.dma_start(xT_sb[:, cc, :], xT[csl, :])
        for cc in range(CC):
            csl = slice(cc * 128, (cc + 1) * 128)
            nc.gpsimd.dma_start(xTb_sb[:, cc, 0:512], xTb[csl, 0:512])
            nc.gpsimd.dma_start(wv_sb[:, cc, :], wv[csl, :])
        for cc in range(CC):
            csl = slice(cc * 128, (cc + 1) * 128)
            nc.gpsimd.dma_start(xTb_sb[:, cc, 512:T], xTb[csl, 512:T])
        wo_sb = w_p.tile([128, PAIRS, C], BF16)
        for jp in range(PAIRS):
            nc.gpsimd.dma_start(wo_sb[:, jp, :], wo[jp * 128:(jp + 1) * 128, :])

        qT = [qk_p.tile([128, T], BF16, tag=f"qT{p}", name=f"qT{p}")
              for p in range(PAIRS)]
        kT = [qk_p.tile([128, T], BF16, tag=f"kT{p}", name=f"kT{p}")
              for p in range(PAIRS)]
        v_sb = v_p.tile([128, TT, HPG, 128], BF16)
        nc.gpsimd.memset(v_sb[:], 0.0)
        for h in range(HPG):
            col = 64 if h % 2 == 0 else 32
            nc.gpsimd.memset(v_sb[:, :, h, col:col + 1], 1.0)
        yT = [y_p.tile([128, T], BF16, tag=f"yT{p}", name=f"yT{p}")
              for p in range(PAIRS)]

        def emit_qkproj(p):
            """Q^T/K^T for pair p: fp8 DoubleRow over cc-pairs, K=256/MM."""
            jsl = slice(p * 128, (p + 1) * 128)
            for w_sb, dst, biased in ((wq_sb, qT[p], True),
                                      (wk_sb, kT[p], False)):
                pss = [small_ps.tile([128, 512], F32, tag="sm",
                                     name=f"pp{p}{biased}{i}")
                       for i in range(QC)]
                for cc2 in range(CC // 2):
                    ksl = slice(2 * cc2, 2 * cc2 + 2)
                    for t in range(QC):
                        nc.tensor.matmul(pss[t][:], w_sb[:, ksl, jsl],
                                         xT_sb[:, ksl, t * 512:(t + 1) * 512],
                                         start=(cc2 == 0),
                                         stop=(cc2 == CC // 2 - 1),
                                         perf_mode=DR)
                for t in range(QC):
                    tsl = slice(t * 512, (t + 1) * 512)
                    if biased:
                        nc.scalar.activation(dst[:, tsl], pss[t][:],
                                             AF.Identity,
                                             bias=bq_sb[:, p:p + 1])
                    else:
                        nc.vector.tensor_copy(dst[:, tsl], pss[t][:])

        def emit_vproj(tt):
            """V rows for t-tile tt into the ones-augmented layout:
            even head (A): cols 0:64 = V, col 64 = 1; odd (B): col 32 = 1,
            cols 64:128 = V."""
            ps = small_ps.tile([128, 512], F32, tag="sm")
            for cc in range(CC):
                nc.tensor.matmul(ps[:], xTb_sb[:, cc, tt * 128:(tt + 1) * 128],
                                 wv_sb[:, cc, :],
                                 start=(cc == 0), stop=(cc == CC - 1))
            psv = ps.rearrange("p (h d) -> p h d", d=D)
            nc.vector.tensor_copy(v_sb[:, tt, 0:HPG:2, 0:D], psv[:, 0:HPG:2, :])
            nc.vector.tensor_copy(v_sb[:, tt, 1:HPG:2, 64:128],
                                  psv[:, 1:HPG:2, :])

        def emit_attention(qc, p):
            qsl = slice(qc * 512, (qc + 1) * 512)
            hA, hB = 2 * p, 2 * p + 1
            nki = 4 * qc + 4
            ya_ps = small_ps.tile([128, 512], F32, tag="sm")
            yb_ps = small_ps.tile([128, 512], F32, tag="sm")
            # boundary strips (r>=0, gpsimd-masked) first: their masking
            # latency overlaps the full strips' exp+PV work
            order = list(range(4 * qc, nki)) + list(range(4 * qc))
            for idx, ki in enumerate(order):
                ksl = slice(ki * 128, (ki + 1) * 128)
                r = ki - 4 * qc
                off = max(0, r) * 128  # first valid tq col of this strip
                qv = slice(qc * 512 + off, (qc + 1) * 512)
                sps = s_ps.tile([128, 1024], F32, tag="s")
                nc.tensor.matmul(sps[:, off:512], kT[p][0:64, ksl],
                                 qT[p][0:64, qv], start=True, stop=True,
                                 tile_position=(0, 0))
                nc.tensor.matmul(sps[:, 512 + off:1024], kT[p][64:128, ksl],
                                 qT[p][64:128, qv], start=True, stop=True,
                                 tile_position=(64, 0))
                et = exp_p.tile([128, 1024], BF16, tag="e")
                if off == 0:
                    nc.scalar.activation(et[:], sps[:], AF.Exp, scale=SCALE)
                else:
                    # one ACT op covering both halves' valid columns via a
                    # two-segment strided AP
                    e2 = et.rearrange("p (h w) -> p h w", h=2)[:, :, off:512]
                    s2 = sps.rearrange("p (h w) -> p h w", h=2)[:, :, off:512]
                    nc.scalar.activation(e2, s2, AF.Exp, scale=SCALE)
                if r >= 0:
                    # zero the causal triangle in-place on GPSIMD: within
                    # the boundary band, col j' is valid iff j' >= p.
                    for base in (off, 512 + off):
                        nc.gpsimd.affine_select(
                            out=et[:, base:base + 128],
                            in_=et[:, base:base + 128],
                            compare_op=ALU.is_ge, fill=0.0, base=0,
                            pattern=[[1, 128]], channel_multiplier=-1)
                mmkw = dict(start=(idx == 0), stop=(idx == nki - 1),
                            skip_group_check=True)
                nc.tensor.matmul(ya_ps[:, off:512], v_sb[:, ki, hA, :],
                                 et[:, off:512], tile_position=(0, 0), **mmkw)
                nc.tensor.matmul(yb_ps[:, off:512], v_sb[:, ki, hB, :],
                                 et[:, 512 + off:1024],
                                 tile_position=(0, 0), **mmkw)
            # 1/L broadcast: two K=1 selector matmuls scatter L_A (psum row
            # 64) to partitions 0:64 and L_B (row 32) to 64:128; reciprocal;
            # the psum->sbuf eviction of y is fused with the normalize mul.
            rec_t = rec_p.tile([65, 512], BF16, tag="rec")
            nc.vector.tensor_copy(rec_t[64:65, :], ya_ps[64:65, :])
            nc.vector.tensor_copy(rec_t[32:33, :], yb_ps[32:33, :])
            bc_ps = small_ps.tile([128, 512], F32, tag="sm")
            nc.tensor.matmul(bc_ps[:], sel_sb[64:65, :], rec_t[64:65, :],
                             start=True, stop=False, skip_group_check=True)
            nc.tensor.matmul(bc_ps[:], sel_sb[32:33, :], rec_t[32:33, :],
                             start=False, stop=True, skip_group_check=True)
            bc_sb = bc_pool.tile([128, 512], F32, tag="bc")
            # rowsums L are strictly positive and well inside normal f32
            # range, so the ~18-bit fast reciprocal is safe
            nc.vector.reciprocal_approx_fast(bc_sb[:], bc_ps[:])
            nc.vector.tensor_mul(yT[p][0:64, qsl], ya_ps[0:64, :],
                                 bc_sb[0:64, :])
            nc.vector.tensor_mul(yT[p][64:128, qsl], yb_ps[64:128, :],
                                 bc_sb[64:128, :])

        def emit_outproj(qc):
            for tt in range(4 * qc, 4 * qc + 4):
                st = stage_p.tile([128, 1024], F32, tag="st")
                for nck in range(2):
                    nsl = slice(nck * 512, (nck + 1) * 512)
                    ops = small_ps.tile([128, 512], F32, tag="sm")
                    for jp in range(PAIRS):
                        nc.tensor.matmul(
                            ops[:], yT[jp][:, tt * 128:(tt + 1) * 128],
                            wo_sb[:, jp, nsl],
                            start=(jp == 0), stop=(jp == PAIRS - 1))
                    nc.vector.tensor_copy(st[:, nsl], ops[:])
                nc.sync.dma_start(out[tt * 128:(tt + 1) * 128, :], st[:])

        # ---- pipelined emission ----
        emit_qkproj(0)
        for tt in range(4):
            emit_vproj(tt)
        emit_attention(0, 0)
        for p in range(1, PAIRS):
            emit_qkproj(p)
            emit_attention(0, p)
        emit_outproj(0)
        for qc in range(1, QC):
            for tt in range(4 * qc, 4 * qc + 4):
                emit_vproj(tt)
            for p in range(PAIRS):
                emit_attention(qc, p)
            emit_outproj(qc)

    nc.compile()
    return nc


_NC_CACHE = None


def _get_nc():
    global _NC_CACHE
    if _NC_CACHE is None:
        _NC_CACHE = build_kernel()
    return _NC_CACHE


def _shard(x, w_qkv, b_qkv, w_out, b_out):
    """Build per-core input maps. Core c: batch c//2, head-group c%2."""
    bf = ml_dtypes.bfloat16
    f8 = ml_dtypes.float8_e4m3
    in_maps = []
    for c in range(N_CORES):
        b, g = divmod(c, 2)
        gs = slice(g * GW, g * GW + GW)
        xt = np.ascontiguousarray(x[b].T)
        in_maps.append({
            "xT": xt.astype(f8),
            "xTb": xt.astype(bf),
            "wq": np.ascontiguousarray(w_qkv[:, gs] * WSCALE).astype(f8),
            "wk": np.ascontiguousarray(
                w_qkv[:, C + g * GW:C + g * GW + GW] * WSCALE).astype(f8),
            "wv": np.ascontiguousarray(
                w_qkv[:, 2 * C + g * GW:2 * C + g * GW + GW]).astype(bf),
            "bq": np.ascontiguousarray(b_qkv[gs] * WSCALE).astype(np.float32),
            "wo": np.ascontiguousarray(w_out[g * GW:g * GW + GW, :]).astype(bf),
        })
    return in_maps


def _unshard(results, b_qkv, w_out, b_out):
    # host-side constant: b_v @ w_out rows (exact: softmax rows sum to 1)
    bv = b_qkv[2 * C:3 * C].astype(np.float64)
    const_row = (bv @ w_out.astype(np.float64)) + b_out.astype(np.float64)
    out = np.empty((B, T, C), dtype=np.float32)
    for b in range(B):
        acc = (results[2 * b]["out"].astype(np.float64)
               + results[2 * b + 1]["out"].astype(np.float64) + const_row)
        out[b] = acc.astype(np.float32)
    return out


def _run(in_maps, trace=False, tmpdir=None):
    nc = _get_nc()
    return run_bass_kernel_spmd(nc, in_maps, core_ids=list(range(N_CORES)),
                                trace=trace, tmpdir=tmpdir)


def kernel(x, w_qkv, b_qkv, w_out, b_out):
    x = np.asarray(x, dtype=np.float32)
    w_qkv = np.asarray(w_qkv, dtype=np.float32)
    b_qkv = np.asarray(b_qkv, dtype=np.float32)
    w_out = np.asarray(w_out, dtype=np.float32)
    b_out = np.asarray(b_out, dtype=np.float32)
    res = _run(_shard(x, w_qkv, b_qkv, w_out, b_out))
    return _unshard(res.results, b_qkv, w_out, b_out)


def kernel_profiled(x, w_qkv, b_qkv, w_out, b_out, tmpdir=None):
    """Like kernel() but captures an NTFF profile (requires the NTFF hook
    to be registered, e.g. via prof_shim.install()). Returns (out, result)."""
    if tmpdir is None:
        tmpdir = tempfile.mkdtemp(prefix="attn_trace_")
    x = np.asarray(x, dtype=np.float32)
    w_qkv = np.asarray(w_qkv, dtype=np.float32)
    b_qkv = np.asarray(b_qkv, dtype=np.float32)
    w_out = np.asarray(w_out, dtype=np.float32)
    b_out = np.asarray(b_out, dtype=np.float32)
    res = _run(_shard(x, w_qkv, b_qkv, w_out, b_out), trace=True,
               tmpdir=tmpdir)
    return _unshard(res.results, b_qkv, w_out, b_out), res


# revision 7
# speedup vs baseline: 1.0828x; 1.0828x over previous
"""Causal self-attention (B=4, T=2048, C=1024, H=16, D=64) on 8 TRN2 cores.

Sharding: core c handles batch b = c//2 and head-group g = c%2 (8 heads).
Each core computes qkv projection for its heads, causal flash attention,
and a partial out-projection (row-parallel over its heads' slice of w_out).
Host sums the two partials per batch and adds biases that commute
(b_k drops under softmax; b_v/b_out fold into a host-side constant row).

Device layout notes:
- x is passed pre-transposed (xT [C, T]) twice: fp8e4 for the Q/K
  projections (fp8 DoubleRow matmuls, K=256/instruction, ~2x bf16) and
  bf16 for the V projection (fp8 anywhere in the P/v/y path blows the
  2e-2 error budget; fp8 q/k only perturbs scores pre-softmax, which
  measures ~1.2e-2). The qkv weights for q/k are pre-scaled by 16 into
  fp8's normal range; S picks up 256x, absorbed into the exp scale, and
  q's bias is pre-multiplied by 16 host-side.
- Q^T/K^T are bf16 in [j, t] layout (head A d in partitions 0:64, head B
  in 64:128) -- exactly the lhsT/rhs layout the attention matmuls need.
- S^T strips [tk=128, tq=512] per head-pair via row-tiled (tile_position)
  K=64 matmuls (the two half-height matmuls stream concurrently), exp'd
  on ACT in one instruction per strip (no max-subtraction: S/8 < ~3),
  causal boundary zeroed in-place by GPSIMD affine_select (condition is
  j' >= p for every diagonal strip). Within each (qc, pair) the boundary
  strips are processed FIRST so their cross-engine masking latency hides
  behind the full strips' exp+PV work instead of stalling the pair's
  rowsum/normalize tail.
- P@V col-tiled into one [128, 512] psum per head (A -> partitions 0:64 +
  L_A at 64 via a ones column; B -> 64:128 + L_B at 32), rowsum broadcast
  via K=1 selector matmuls, reciprocal on DVE, and the psum->sbuf y
  eviction fused with the 1/L normalize multiply.
- Emission order pipelines load -> projection -> attention: qk-proj(p0),
  V(tt0:4), attn(qc0,p0), qk-proj(p1), attn(qc0,p1), ... so ACT starts
  exp'ing ~10us in instead of after all projections.
"""

import tempfile
from contextlib import ExitStack

import numpy as np
import ml_dtypes

import concourse.bass as bass
import concourse.tile as tile
from concourse import bacc, mybir
from concourse.bass_utils import run_bass_kernel_spmd

BF16 = mybir.dt.bfloat16
F32 = mybir.dt.float32
FP8 = mybir.dt.float8e4
AF = mybir.ActivationFunctionType
ALU = mybir.AluOpType
DR = mybir.MatmulPerfMode.DoubleRow

B, T, C, H, D = 4, 2048, 1024, 16, 64
HPG = 8                 # heads per core (group)
PAIRS = HPG // 2        # head pairs per core
GW = HPG * D            # 512: group width of q/k/v
CC = C // 128           # 8 contraction chunks
TT = T // 128           # 16 t-tiles
QC = T // 512           # 4 query chunks of 512
WSCALE = 16.0           # host-side q/k weight scale (fp8 subnormal dodge)
SCALE = 1.0 / (np.sqrt(D) * WSCALE * WSCALE)   # exp scale: S_raw -> S/8
N_CORES = 8


def build_kernel():
    nc = bacc.Bacc("TRN2", target_bir_lowering=False, debug=False,
                   num_devices=N_CORES)
    xT = nc.dram_tensor("xT", [C, T], FP8, kind="ExternalInput").ap()
    xTb = nc.dram_tensor("xTb", [C, T], BF16, kind="ExternalInput").ap()
    wqk = nc.dram_tensor("wqk", [C, 2, GW], FP8, kind="ExternalInput").ap()
    wv = nc.dram_tensor("wv", [C, GW], BF16, kind="ExternalInput").ap()
    bq = nc.dram_tensor("bq", [GW], F32, kind="ExternalInput").ap()
    wo = nc.dram_tensor("wo", [GW, C], BF16, kind="ExternalInput").ap()
    out = nc.dram_tensor("out", [T, C], F32, kind="ExternalOutput").ap()

    with tile.TileContext(nc) as tc, ExitStack() as ctx:
        const_p = ctx.enter_context(tc.tile_pool(name="const", bufs=1))
        w_p = ctx.enter_context(tc.tile_pool(name="w", bufs=1))
        x_p = ctx.enter_context(tc.tile_pool(name="x", bufs=1))
        qk_p = ctx.enter_context(tc.tile_pool(name="qk", bufs=1))
        v_p = ctx.enter_context(tc.tile_pool(name="v", bufs=1))
        y_p = ctx.enter_context(tc.tile_pool(name="y", bufs=1))
        exp_p = ctx.enter_context(tc.tile_pool(name="expt", bufs=8))
        stage_p = ctx.enter_context(tc.tile_pool(name="stage", bufs=3))
        # PSUM budget (8 banks): s_ps 2x[128,1024]=4, small_ps 4x[128,512]=4
        s_ps = ctx.enter_context(tc.tile_pool(name="s_ps", bufs=2, space="PSUM"))
        small_ps = ctx.enter_context(
            tc.tile_pool(name="small_ps", bufs=4, space="PSUM"))
        rec_p = ctx.enter_context(tc.tile_pool(name="rec", bufs=4))
        bc_pool = ctx.enter_context(tc.tile_pool(name="bc_sb", bufs=4))

        # ---- constants ----
        # selector for broadcasting rowsums via PE: row64 (head A's L)
        # -> out partitions 0:64, row32 (head B's L) -> 64:128.
        sel_sb = const_p.tile([65, 128], BF16)
        nc.gpsimd.memset(sel_sb[:], 0.0)
        nc.gpsimd.memset(sel_sb[64:65, 0:64], 1.0)
        nc.gpsimd.memset(sel_sb[32:33, 64:128], 1.0)

        # ---- loads, in dependency-criticality order ----
        # sync queue: bq, then per-cc {wq, wk, xT8} (Q/K proj inputs).
        # vector queue: xTb t-chunk 0 + wv (V proj for tt 0:4), then the
        # rest of xTb, then wo (needed first at outproj(qc0)).
        bq_sb = w_p.tile([128, PAIRS], F32)
        nc.sync.dma_start(bq_sb[:], bq.rearrange("(p c) -> c p", c=128))
        wqk_sb = w_p.tile([128, CC, 2, GW], FP8)
        wv_sb = w_p.tile([128, CC, GW], BF16)
        xT_sb = x_p.tile([128, CC, T], FP8)
        xTb_sb = x_p.tile([128, CC, T], BF16)
        xT_r = xT.rearrange("(cc p) t -> p cc t", p=128)
        wqk_r = wqk.rearrange("(cc p) two g -> p cc two g", p=128)
        xTb_r = xTb.rearrange("(cc p) t -> p cc t", p=128)
        # few big strided DMAs: issue cost on the queueing engine is ~600ns
        # per DMA, so granularity is halves, not per-cc chunks
        nc.sync.dma_start(wqk_sb[:, 0:4], wqk_r[:, 0:4])
        nc.sync.dma_start(xT_sb[:, 0:4, :], xT_r[:, 0:4, :])
        nc.sync.dma_start(wqk_sb[:, 4:8], wqk_r[:, 4:8])
        nc.sync.dma_start(xT_sb[:, 4:8, :], xT_r[:, 4:8, :])
        nc.gpsimd.dma_start(xTb_sb[:, :, 0:1024], xTb_r[:, :, 0:1024])
        nc.gpsimd.dma_start(wv_sb[:],
                            wv.rearrange("(cc p) g -> p cc g", p=128))
        nc.gpsimd.dma_start(xTb_sb[:, :, 1024:T], xTb_r[:, :, 1024:T])
        wo_sb = w_p.tile([128, PAIRS, C], BF16)
        nc.gpsimd.dma_start(wo_sb[:],
                            wo.rearrange("(jp p) c -> p jp c", p=128))

        # causal boundary mask, two identical [128,128] triangles side by
        # side (mask2[p, s, j] = 1 iff j >= p) for one 2-segment DVE
        # multiply per boundary strip
        mask2_sb = const_p.tile([128, 2, 128], BF16)
        nc.gpsimd.memset(mask2_sb[:], 1.0)
        for s in range(2):
            nc.gpsimd.affine_select(
                out=mask2_sb[:, s, :], in_=mask2_sb[:, s, :],
                compare_op=ALU.is_ge, fill=0.0, base=0,
                pattern=[[1, 128]], channel_multiplier=-1)

        qT = [qk_p.tile([128, T], BF16, tag=f"qT{p}", name=f"qT{p}")
              for p in range(PAIRS)]
        kT = [qk_p.tile([128, T], BF16, tag=f"kT{p}", name=f"kT{p}")
              for p in range(PAIRS)]
        v_sb = v_p.tile([128, TT, HPG, 128], BF16)

        def emit_vinit(g):
            """Zero the pad columns + set the ones columns of v_sb for
            t-tiles [4g, 4g+4), on DVE (cheap, and off the gpsimd queue)."""
            tsl = slice(4 * g, 4 * g + 4)
            nc.vector.memset(v_sb[:, tsl, 0:HPG:2, 64:128], 0.0)
            nc.vector.memset(v_sb[:, tsl, 1:HPG:2, 0:64], 0.0)
            for h in range(HPG):
                col = 64 if h % 2 == 0 else 32
                nc.vector.memset(v_sb[:, tsl, h, col:col + 1], 1.0)

        yT = [y_p.tile([128, T], BF16, tag=f"yT{p}", name=f"yT{p}")
              for p in range(PAIRS)]

        def emit_qkproj(p):
            """Q^T/K^T for pair p: fp8 DoubleRow over cc-pairs, K=256/MM."""
            jsl = slice(p * 128, (p + 1) * 128)
            for qk_i, dst, biased in ((0, qT[p], True), (1, kT[p], False)):
                pss = [small_ps.tile([128, 512], F32, tag="sm",
                                     name=f"pp{p}{biased}{i}")
                       for i in range(QC)]
                for cc2 in range(CC // 2):
                    ksl = slice(2 * cc2, 2 * cc2 + 2)
                    for t in range(QC):
                        nc.tensor.matmul(pss[t][:], wqk_sb[:, ksl, qk_i, jsl],
                                         xT_sb[:, ksl, t * 512:(t + 1) * 512],
                                         start=(cc2 == 0),
                                         stop=(cc2 == CC // 2 - 1),
                                         perf_mode=DR)
                for t in range(QC):
                    tsl = slice(t * 512, (t + 1) * 512)
                    if biased:
                        nc.scalar.activation(dst[:, tsl], pss[t][:],
                                             AF.Identity,
                                             bias=bq_sb[:, p:p + 1])
                    else:
                        nc.vector.tensor_copy(dst[:, tsl], pss[t][:])

        def emit_vproj(tt):
            """V rows for t-tile tt into the ones-augmented layout:
            even head (A): cols 0:64 = V, col 64 = 1; odd (B): col 32 = 1,
            cols 64:128 = V."""
            ps = small_ps.tile([128, 512], F32, tag="sm")
            for cc in range(CC):
                nc.tensor.matmul(ps[:], xTb_sb[:, cc, tt * 128:(tt + 1) * 128],
                                 wv_sb[:, cc, :],
                                 start=(cc == 0), stop=(cc == CC - 1))
            psv = ps.rearrange("p (h d) -> p h d", d=D)
            nc.vector.tensor_copy(v_sb[:, tt, 0:HPG:2, 0:D], psv[:, 0:HPG:2, :])
            nc.vector.tensor_copy(v_sb[:, tt, 1:HPG:2, 64:128],
                                  psv[:, 1:HPG:2, :])

        def emit_attention(qc, p):
            qsl = slice(qc * 512, (qc + 1) * 512)
            hA, hB = 2 * p, 2 * p + 1
            nki = 4 * qc + 4
            ya_ps = small_ps.tile([128, 512], F32, tag="sm")
            yb_ps = small_ps.tile([128, 512], F32, tag="sm")
            # boundary strips (r>=0, gpsimd-masked) first: their masking
            # latency overlaps the full strips' exp+PV work
            order = list(range(4 * qc, nki)) + list(range(4 * qc))
            for idx, ki in enumerate(order):
                ksl = slice(ki * 128, (ki + 1) * 128)
                r = ki - 4 * qc
                off = max(0, r) * 128  # first valid tq col of this strip
                qv = slice(qc * 512 + off, (qc + 1) * 512)
                sps = s_ps.tile([128, 1024], F32, tag="s")
                nc.tensor.matmul(sps[:, off:512], kT[p][0:64, ksl],
                                 qT[p][0:64, qv], start=True, stop=True,
                                 tile_position=(0, 0))
                nc.tensor.matmul(sps[:, 512 + off:1024], kT[p][64:128, ksl],
                                 qT[p][64:128, qv], start=True, stop=True,
                                 tile_position=(64, 0))
                et = exp_p.tile([128, 1024], BF16, tag="e")
                if off == 0:
                    nc.scalar.activation(et[:], sps[:], AF.Exp, scale=SCALE)
                else:
                    # one ACT op covering both halves' valid columns via a
                    # two-segment strided AP
                    e2 = et.rearrange("p (h w) -> p h w", h=2)[:, :, off:512]
                    s2 = sps.rearrange("p (h w) -> p h w", h=2)[:, :, off:512]
                    nc.scalar.activation(e2, s2, AF.Exp, scale=SCALE)
                if r >= 0:
                    # zero the causal triangle: one 2-segment DVE multiply
                    # covering both heads' boundary bands
                    e_m = et.rearrange("p (h w) -> p h w",
                                       h=2)[:, :, off:off + 128]
                    nc.vector.tensor_mul(e_m, e_m, mask2_sb[:])
                mmkw = dict(start=(idx == 0), stop=(idx == nki - 1),
                            skip_group_check=True)
                nc.tensor.matmul(ya_ps[:, off:512], v_sb[:, ki, hA, :],
                                 et[:, off:512], tile_position=(0, 0), **mmkw)
                nc.tensor.matmul(yb_ps[:, off:512], v_sb[:, ki, hB, :],
                                 et[:, 512 + off:1024],
                                 tile_position=(0, 0), **mmkw)
            # 1/L broadcast: two K=1 selector matmuls scatter L_A (psum row
            # 64) to partitions 0:64 and L_B (row 32) to 64:128; reciprocal;
            # the psum->sbuf eviction of y is fused with the normalize mul.
            rec_t = rec_p.tile([65, 512], BF16, tag="rec")
            nc.vector.tensor_copy(rec_t[64:65, :], ya_ps[64:65, :])
            nc.vector.tensor_copy(rec_t[32:33, :], yb_ps[32:33, :])
            bc_ps = small_ps.tile([128, 512], F32, tag="sm")
            nc.tensor.matmul(bc_ps[:], sel_sb[64:65, :], rec_t[64:65, :],
                             start=True, stop=False, skip_group_check=True)
            nc.tensor.matmul(bc_ps[:], sel_sb[32:33, :], rec_t[32:33, :],
                             start=False, stop=True, skip_group_check=True)
            bc_sb = bc_pool.tile([128, 512], F32, tag="bc")
            # rowsums L are strictly positive and well inside normal f32
            # range, so the ~18-bit fast reciprocal is safe
            nc.vector.reciprocal_approx_fast(bc_sb[:], bc_ps[:])
            nc.vector.tensor_mul(yT[p][0:64, qsl], ya_ps[0:64, :],
                                 bc_sb[0:64, :])
            nc.vector.tensor_mul(yT[p][64:128, qsl], yb_ps[64:128, :],
                                 bc_sb[64:128, :])

        def emit_outproj(qc):
            for tt in range(4 * qc, 4 * qc + 4):
                st = stage_p.tile([128, 1024], F32, tag="st")
                for nck in range(2):
                    nsl = slice(nck * 512, (nck + 1) * 512)
                    ops = small_ps.tile([128, 512], F32, tag="sm")
                    for jp in range(PAIRS):
                        nc.tensor.matmul(
                            ops[:], yT[jp][:, tt * 128:(tt + 1) * 128],
                            wo_sb[:, jp, nsl],
                            start=(jp == 0), stop=(jp == PAIRS - 1))
                    nc.vector.tensor_copy(st[:, nsl], ops[:])
                nc.sync.dma_start(out[tt * 128:(tt + 1) * 128, :], st[:])

        # ---- pipelined emission ----
        emit_vinit(0)
        emit_qkproj(0)
        for tt in range(4):
            emit_vproj(tt)
        emit_attention(0, 0)
        for p in range(1, PAIRS):
            emit_qkproj(p)
            emit_attention(0, p)
        emit_vinit(1)
        emit_outproj(0)
        for qc in range(1, QC):
            for tt in range(4 * qc, 4 * qc + 4):
                emit_vproj(tt)
            if qc < QC - 1:
                emit_vinit(qc + 1)
            for p in range(PAIRS):
                emit_attention(qc, p)
            emit_outproj(qc)

    nc.compile()
    return nc


_NC_CACHE = None


def _get_nc():
    global _NC_CACHE
    if _NC_CACHE is None:
        _NC_CACHE = build_kernel()
    return _NC_CACHE


def _shard(x, w_qkv, b_qkv, w_out, b_out):
    """Build per-core input maps. Core c: batch c//2, head-group c%2."""
    bf = ml_dtypes.bfloat16
    f8 = ml_dtypes.float8_e4m3
    in_maps = []
    for c in range(N_CORES):
        b, g = divmod(c, 2)
        gs = slice(g * GW, g * GW + GW)
        xt = np.ascontiguousarray(x[b].T)
        in_maps.append({
            "xT": xt.astype(f8),
            "xTb": xt.astype(bf),
            "wqk": np.ascontiguousarray(np.stack(
                [w_qkv[:, gs] * WSCALE,
                 w_qkv[:, C + g * GW:C + g * GW + GW] * WSCALE],
                axis=1)).astype(f8),
            "wv": np.ascontiguousarray(
                w_qkv[:, 2 * C + g * GW:2 * C + g * GW + GW]).astype(bf),
            "bq": np.ascontiguousarray(b_qkv[gs] * WSCALE).astype(np.float32),
            "wo": np.ascontiguousarray(w_out[g * GW:g * GW + GW, :]).astype(bf),
        })
    return in_maps


def _unshard(results, b_qkv, w_out, b_out):
    # host-side constant: b_v @ w_out rows (exact: softmax rows sum to 1)
    bv = b_qkv[2 * C:3 * C].astype(np.float64)
    const_row = (bv @ w_out.astype(np.float64)) + b_out.astype(np.float64)
    out = np.empty((B, T, C), dtype=np.float32)
    for b in range(B):
        acc = (results[2 * b]["out"].astype(np.float64)
               + results[2 * b + 1]["out"].astype(np.float64) + const_row)
        out[b] = acc.astype(np.float32)
    return out


def _run(in_maps, trace=False, tmpdir=None):
    nc = _get_nc()
    return run_bass_kernel_spmd(nc, in_maps, core_ids=list(range(N_CORES)),
                                trace=trace, tmpdir=tmpdir)


def kernel(x, w_qkv, b_qkv, w_out, b_out):
    x = np.asarray(x, dtype=np.float32)
    w_qkv = np.asarray(w_qkv, dtype=np.float32)
    b_qkv = np.asarray(b_qkv, dtype=np.float32)
    w_out = np.asarray(w_out, dtype=np.float32)
    b_out = np.asarray(b_out, dtype=np.float32)
    res = _run(_shard(x, w_qkv, b_qkv, w_out, b_out))
    return _unshard(res.results, b_qkv, w_out, b_out)


def kernel_profiled(x, w_qkv, b_qkv, w_out, b_out, tmpdir=None):
    """Like kernel() but captures an NTFF profile (requires the NTFF hook
    to be registered, e.g. via prof_shim.install()). Returns (out, result)."""
    if tmpdir is None:
        tmpdir = tempfile.mkdtemp(prefix="attn_trace_")
    x = np.asarray(x, dtype=np.float32)
    w_qkv = np.asarray(w_qkv, dtype=np.float32)
    b_qkv = np.asarray(b_qkv, dtype=np.float32)
    w_out = np.asarray(w_out, dtype=np.float32)
    b_out = np.asarray(b_out, dtype=np.float32)
    res = _run(_shard(x, w_qkv, b_qkv, w_out, b_out), trace=True,
               tmpdir=tmpdir)
    return _unshard(res.results, b_qkv, w_out, b_out), res


# revision 10
# speedup vs baseline: 1.0840x; 1.0011x over previous
"""Causal self-attention (B=4, T=2048, C=1024, H=16, D=64) on 8 TRN2 cores.

Sharding: core c handles batch b = c//2 and head-group g = c%2 (8 heads).
Each core computes qkv projection for its heads, causal flash attention,
and a partial out-projection (row-parallel over its heads' slice of w_out).
Host sums the two partials per batch and adds biases that commute
(b_k drops under softmax; b_v/b_out fold into a host-side constant row).

Device layout notes:
- x is passed pre-transposed (xT [C, T]) twice: fp8e4 for the Q/K
  projections (fp8 DoubleRow matmuls, K=256/instruction, ~2x bf16) and
  bf16 for the V projection (fp8 anywhere in the P/v/y path blows the
  2e-2 error budget; fp8 q/k only perturbs scores pre-softmax, which
  measures ~1.2e-2). The qkv weights for q/k are pre-scaled by 16 into
  fp8's normal range; S picks up 256x, absorbed into the exp scale, and
  q's bias is pre-multiplied by 16 host-side.
- Q^T/K^T are bf16 in [j, t] layout (head A d in partitions 0:64, head B
  in 64:128) -- exactly the lhsT/rhs layout the attention matmuls need.
- S^T strips [tk=128, tq=512] per head-pair via row-tiled (tile_position)
  K=64 matmuls (the two half-height matmuls stream concurrently), exp'd
  on ACT in one instruction per strip (no max-subtraction: S/8 < ~3),
  causal boundary zeroed in-place by GPSIMD affine_select (condition is
  j' >= p for every diagonal strip). Within each (qc, pair) the boundary
  strips are processed FIRST so their cross-engine masking latency hides
  behind the full strips' exp+PV work instead of stalling the pair's
  rowsum/normalize tail.
- P@V col-tiled into one [128, 512] psum per head (A -> partitions 0:64 +
  L_A at 64 via a ones column; B -> 64:128 + L_B at 32), rowsum broadcast
  via K=1 selector matmuls, reciprocal on DVE, and the psum->sbuf y
  eviction fused with the 1/L normalize multiply.
- Emission order pipelines load -> projection -> attention: qk-proj(p0),
  V(tt0:4), attn(qc0,p0), qk-proj(p1), attn(qc0,p1), ... so ACT starts
  exp'ing ~10us in instead of after all projections.
"""

import tempfile
from contextlib import ExitStack

import numpy as np
import ml_dtypes

import concourse.bass as bass
import concourse.tile as tile
from concourse import bacc, mybir
from concourse.bass_utils import run_bass_kernel_spmd

BF16 = mybir.dt.bfloat16
F32 = mybir.dt.float32
FP8 = mybir.dt.float8e4
AF = mybir.ActivationFunctionType
ALU = mybir.AluOpType
DR = mybir.MatmulPerfMode.DoubleRow

B, T, C, H, D = 4, 2048, 1024, 16, 64
HPG = 8                 # heads per core (group)
PAIRS = HPG // 2        # head pairs per core
GW = HPG * D            # 512: group width of q/k/v
CC = C // 128           # 8 contraction chunks
TT = T // 128           # 16 t-tiles
QC = T // 512           # 4 query chunks of 512
WSCALE = 16.0           # host-side q/k weight scale (fp8 subnormal dodge)
SCALE = 1.0 / (np.sqrt(D) * WSCALE * WSCALE)   # exp scale: S_raw -> S/8
N_CORES = 8


def build_kernel():
    nc = bacc.Bacc("TRN2", target_bir_lowering=False, debug=False,
                   num_devices=N_CORES)
    xT = nc.dram_tensor("xT", [C, T], FP8, kind="ExternalInput").ap()
    xTb = nc.dram_tensor("xTb", [C, T], BF16, kind="ExternalInput").ap()
    wqk = nc.dram_tensor("wqk", [C, 2, GW], FP8, kind="ExternalInput").ap()
    wv = nc.dram_tensor("wv", [C, GW], BF16, kind="ExternalInput").ap()
    bq = nc.dram_tensor("bq", [GW], F32, kind="ExternalInput").ap()
    wo = nc.dram_tensor("wo", [GW, C], BF16, kind="ExternalInput").ap()
    out = nc.dram_tensor("out", [T, C], F32, kind="ExternalOutput").ap()

    with tile.TileContext(nc) as tc, ExitStack() as ctx:
        const_p = ctx.enter_context(tc.tile_pool(name="const", bufs=1))
        w_p = ctx.enter_context(tc.tile_pool(name="w", bufs=1))
        x_p = ctx.enter_context(tc.tile_pool(name="x", bufs=1))
        qk_p = ctx.enter_context(tc.tile_pool(name="qk", bufs=1))
        v_p = ctx.enter_context(tc.tile_pool(name="v", bufs=1))
        y_p = ctx.enter_context(tc.tile_pool(name="y", bufs=1))
        exp_p = ctx.enter_context(tc.tile_pool(name="expt", bufs=8))
        stage_p = ctx.enter_context(tc.tile_pool(name="stage", bufs=3))
        # PSUM budget (8 banks): s_ps 2x[128,1024]=4, small_ps 4x[128,512]=4
        s_ps = ctx.enter_context(tc.tile_pool(name="s_ps", bufs=2, space="PSUM"))
        small_ps = ctx.enter_context(
            tc.tile_pool(name="small_ps", bufs=4, space="PSUM"))
        rec_p = ctx.enter_context(tc.tile_pool(name="rec", bufs=4))
        bc_pool = ctx.enter_context(tc.tile_pool(name="bc_sb", bufs=4))

        # ---- loads, in dependency-criticality order ----
        # sync queue: bq, then per-cc {wq, wk, xT8} (Q/K proj inputs).
        # vector queue: xTb t-chunk 0 + wv (V proj for tt 0:4), then the
        # rest of xTb, then wo (needed first at outproj(qc0)).
        bq_sb = w_p.tile([128, PAIRS], F32)
        nc.sync.dma_start(bq_sb[:], bq.rearrange("(p c) -> c p", c=128))
        wqk_sb = w_p.tile([128, CC, 2, GW], FP8)
        wv_sb = w_p.tile([128, CC, GW], BF16)
        xT_sb = x_p.tile([128, CC, T], FP8)
        xTb_sb = x_p.tile([128, CC, T], BF16)
        # per-cc simple 2D DMAs (keeps them on the hardware DGE; the
        # rearranged multi-MB variants fall back to software DGE at a
        # fraction of the bandwidth), split across the sync/scalar/gpsimd
        # issue queues
        for cc in range(CC):
            csl = slice(cc * 128, (cc + 1) * 128)
            nc.sync.dma_start(wqk_sb[:, cc], wqk[csl])
            nc.scalar.dma_start(xT_sb[:, cc, :], xT[csl, :])
        wo_sb = w_p.tile([128, PAIRS, C], BF16)
        for cc in range(CC):
            csl = slice(cc * 128, (cc + 1) * 128)
            nc.gpsimd.dma_start(xTb_sb[:, cc, :], xTb[csl, :])
            nc.gpsimd.dma_start(wv_sb[:, cc, :], wv[csl, :])
        for jp in range(PAIRS):
            nc.gpsimd.dma_start(wo_sb[:, jp, :], wo[jp * 128:(jp + 1) * 128, :])

        # selector for broadcasting rowsums via PE: row64 (head A's L)
        # -> out partitions 0:64, row32 (head B's L) -> 64:128.
        sel_sb = const_p.tile([65, 128], BF16)
        nc.gpsimd.memset(sel_sb[:], 0.0)
        nc.gpsimd.memset(sel_sb[64:65, 0:64], 1.0)
        nc.gpsimd.memset(sel_sb[32:33, 64:128], 1.0)
        # causal boundary mask, two identical [128,128] triangles side by
        # side (mask2[p, s, j] = 1 iff j >= p) for one 2-segment DVE
        # multiply per boundary strip
        mask2_sb = const_p.tile([128, 2, 128], BF16)
        nc.gpsimd.memset(mask2_sb[:], 1.0)
        for s in range(2):
            nc.gpsimd.affine_select(
                out=mask2_sb[:, s, :], in_=mask2_sb[:, s, :],
                compare_op=ALU.is_ge, fill=0.0, base=0,
                pattern=[[1, 128]], channel_multiplier=-1)

        qT = [qk_p.tile([128, T], BF16, tag=f"qT{p}", name=f"qT{p}")
              for p in range(PAIRS)]
        kT = [qk_p.tile([128, T], BF16, tag=f"kT{p}", name=f"kT{p}")
              for p in range(PAIRS)]
        v_sb = v_p.tile([128, TT, HPG, 128], BF16)

        def emit_vinit(g):
            """Zero the pad columns + set the ones columns of v_sb for
            t-tiles [4g, 4g+4), on DVE (cheap, and off the gpsimd queue)."""
            tsl = slice(4 * g, 4 * g + 4)
            nc.vector.memset(v_sb[:, tsl, 0:HPG:2, 64:128], 0.0)
            nc.vector.memset(v_sb[:, tsl, 1:HPG:2, 0:64], 0.0)
            for h in range(HPG):
                col = 64 if h % 2 == 0 else 32
                nc.vector.memset(v_sb[:, tsl, h, col:col + 1], 1.0)

        yT = [y_p.tile([128, T], BF16, tag=f"yT{p}", name=f"yT{p}")
              for p in range(PAIRS)]

        def emit_qkproj(p):
            """Q^T/K^T for pair p: fp8 DoubleRow over cc-pairs, K=256/MM."""
            jsl = slice(p * 128, (p + 1) * 128)
            for qk_i, dst, biased in ((0, qT[p], True), (1, kT[p], False)):
                pss = [small_ps.tile([128, 512], F32, tag="sm",
                                     name=f"pp{p}{biased}{i}")
                       for i in range(QC)]
                for cc2 in range(CC // 2):
                    ksl = slice(2 * cc2, 2 * cc2 + 2)
                    for t in range(QC):
                        nc.tensor.matmul(pss[t][:], wqk_sb[:, ksl, qk_i, jsl],
                                         xT_sb[:, ksl, t * 512:(t + 1) * 512],
                                         start=(cc2 == 0),
                                         stop=(cc2 == CC // 2 - 1),
                                         perf_mode=DR)
                for t in range(QC):
                    tsl = slice(t * 512, (t + 1) * 512)
                    if biased:
                        nc.scalar.activation(dst[:, tsl], pss[t][:],
                                             AF.Identity,
                                             bias=bq_sb[:, p:p + 1])
                    else:
                        nc.vector.tensor_copy(dst[:, tsl], pss[t][:])

        def emit_vproj(tt):
            """V rows for t-tile tt into the ones-augmented layout:
            even head (A): cols 0:64 = V, col 64 = 1; odd (B): col 32 = 1,
            cols 64:128 = V."""
            ps = small_ps.tile([128, 512], F32, tag="sm")
            for cc in range(CC):
                nc.tensor.matmul(ps[:], xTb_sb[:, cc, tt * 128:(tt + 1) * 128],
                                 wv_sb[:, cc, :],
                                 start=(cc == 0), stop=(cc == CC - 1))
            psv = ps.rearrange("p (h d) -> p h d", d=D)
            nc.vector.tensor_copy(v_sb[:, tt, 0:HPG:2, 0:D], psv[:, 0:HPG:2, :])
            nc.vector.tensor_copy(v_sb[:, tt, 1:HPG:2, 64:128],
                                  psv[:, 1:HPG:2, :])

        def emit_attention(qc, p):
            qsl = slice(qc * 512, (qc + 1) * 512)
            hA, hB = 2 * p, 2 * p + 1
            nki = 4 * qc + 4
            ya_ps = small_ps.tile([128, 512], F32, tag="sm")
            yb_ps = small_ps.tile([128, 512], F32, tag="sm")
            # boundary strips (r>=0, gpsimd-masked) first: their masking
            # latency overlaps the full strips' exp+PV work
            order = list(range(4 * qc, nki)) + list(range(4 * qc))
            for idx, ki in enumerate(order):
                ksl = slice(ki * 128, (ki + 1) * 128)
                r = ki - 4 * qc
                off = max(0, r) * 128  # first valid tq col of this strip
                qv = slice(qc * 512 + off, (qc + 1) * 512)
                sps = s_ps.tile([128, 1024], F32, tag="s")
                nc.tensor.matmul(sps[:, off:512], kT[p][0:64, ksl],
                                 qT[p][0:64, qv], start=True, stop=True,
                                 tile_position=(0, 0))
                nc.tensor.matmul(sps[:, 512 + off:1024], kT[p][64:128, ksl],
                                 qT[p][64:128, qv], start=True, stop=True,
                                 tile_position=(64, 0))
                et = exp_p.tile([128, 1024], BF16, tag="e")
                if off == 0:
                    nc.scalar.activation(et[:], sps[:], AF.Exp, scale=SCALE)
                else:
                    # one ACT op covering both halves' valid columns via a
                    # two-segment strided AP
                    e2 = et.rearrange("p (h w) -> p h w", h=2)[:, :, off:512]
                    s2 = sps.rearrange("p (h w) -> p h w", h=2)[:, :, off:512]
                    nc.scalar.activation(e2, s2, AF.Exp, scale=SCALE)
                if r >= 0:
                    # zero the causal triangle: one 2-segment DVE multiply
                    # covering both heads' boundary bands
                    e_m = et.rearrange("p (h w) -> p h w",
                                       h=2)[:, :, off:off + 128]
                    nc.vector.tensor_mul(e_m, e_m, mask2_sb[:])
                mmkw = dict(start=(idx == 0), stop=(idx == nki - 1),
                            skip_group_check=True)
                nc.tensor.matmul(ya_ps[:, off:512], v_sb[:, ki, hA, :],
                                 et[:, off:512], tile_position=(0, 0), **mmkw)
                nc.tensor.matmul(yb_ps[:, off:512], v_sb[:, ki, hB, :],
                                 et[:, 512 + off:1024],
                                 tile_position=(0, 0), **mmkw)
            # 1/L broadcast: two K=1 selector matmuls scatter L_A (psum row
            # 64) to partitions 0:64 and L_B (row 32) to 64:128; reciprocal;
            # the psum->sbuf eviction of y is fused with the normalize mul.
            rec_t = rec_p.tile([65, 512], BF16, tag="rec")
            nc.vector.tensor_copy(rec_t[64:65, :], ya_ps[64:65, :])
            nc.vector.tensor_copy(rec_t[32:33, :], yb_ps[32:33, :])
            bc_ps = small_ps.tile([128, 512], F32, tag="sm")
            nc.tensor.matmul(bc_ps[:], sel_sb[64:65, :], rec_t[64:65, :],
                             start=True, stop=False, skip_group_check=True)
            nc.tensor.matmul(bc_ps[:], sel_sb[32:33, :], rec_t[32:33, :],
                             start=False, stop=True, skip_group_check=True)
            bc_sb = bc_pool.tile([128, 512], F32, tag="bc")
            # rowsums L are strictly positive and well inside normal f32
            # range, so the ~18-bit fast reciprocal is safe
            nc.vector.reciprocal_approx_fast(bc_sb[:], bc_ps[:])
            nc.vector.tensor_mul(yT[p][0:64, qsl], ya_ps[0:64, :],
                                 bc_sb[0:64, :])
            nc.vector.tensor_mul(yT[p][64:128, qsl], yb_ps[64:128, :],
                                 bc_sb[64:128, :])

        def emit_outproj(qc):
            for tt in range(4 * qc, 4 * qc + 4):
                st = stage_p.tile([128, 1024], F32, tag="st")
                for nck in range(2):
                    nsl = slice(nck * 512, (nck + 1) * 512)
                    ops = small_ps.tile([128, 512], F32, tag="sm")
                    for jp in range(PAIRS):
                        nc.tensor.matmul(
                            ops[:], yT[jp][:, tt * 128:(tt + 1) * 128],
                            wo_sb[:, jp, nsl],
                            start=(jp == 0), stop=(jp == PAIRS - 1))
                    nc.vector.tensor_copy(st[:, nsl], ops[:])
                nc.sync.dma_start(out[tt * 128:(tt + 1) * 128, :], st[:])

        # ---- pipelined emission ----
        emit_vinit(0)
        emit_qkproj(0)
        for tt in range(4):
            emit_vproj(tt)
        emit_attention(0, 0)
        for p in range(1, PAIRS):
            emit_qkproj(p)
            emit_attention(0, p)
        emit_vinit(1)
        emit_outproj(0)
        for qc in range(1, QC):
            for tt in range(4 * qc, 4 * qc + 4):
                emit_vproj(tt)
            if qc < QC - 1:
                emit_vinit(qc + 1)
            for p in range(PAIRS):
                emit_attention(qc, p)
            emit_outproj(qc)

    nc.compile()
    return nc


_NC_CACHE = None


def _get_nc():
    global _NC_CACHE
    if _NC_CACHE is None:
        _NC_CACHE = build_kernel()
    return _NC_CACHE


def _shard(x, w_qkv, b_qkv, w_out, b_out):
    """Build per-core input maps. Core c: batch c//2, head-group c%2."""
    bf = ml_dtypes.bfloat16
    f8 = ml_dtypes.float8_e4m3
    in_maps = []
    for c in range(N_CORES):
        b, g = divmod(c, 2)
        gs = slice(g * GW, g * GW + GW)
        xt = np.ascontiguousarray(x[b].T)
        in_maps.append({
            "xT": xt.astype(f8),
            "xTb": xt.astype(bf),
            "wqk": np.ascontiguousarray(np.stack(
                [w_qkv[:, gs] * WSCALE,
                 w_qkv[:, C + g * GW:C + g * GW + GW] * WSCALE],
                axis=1)).astype(f8),
            "wv": np.ascontiguousarray(
                w_qkv[:, 2 * C + g * GW:2 * C + g * GW + GW]).astype(bf),
            "bq": np.ascontiguousarray(b_qkv[gs] * WSCALE).astype(np.float32),
            "wo": np.ascontiguousarray(w_out[g * GW:g * GW + GW, :]).astype(bf),
        })
    return in_maps


def _unshard(results, b_qkv, w_out, b_out):
    # host-side constant: b_v @ w_out rows (exact: softmax rows sum to 1)
    bv = b_qkv[2 * C:3 * C].astype(np.float64)
    const_row = (bv @ w_out.astype(np.float64)) + b_out.astype(np.float64)
    out = np.empty((B, T, C), dtype=np.float32)
    for b in range(B):
        acc = (results[2 * b]["out"].astype(np.float64)
               + results[2 * b + 1]["out"].astype(np.float64) + const_row)
        out[b] = acc.astype(np.float32)
    return out


def _run(in_maps, trace=False, tmpdir=None):
    nc = _get_nc()
    return run_bass_kernel_spmd(nc, in_maps, core_ids=list(range(N_CORES)),
                                trace=trace, tmpdir=tmpdir)


def kernel(x, w_qkv, b_qkv, w_out, b_out):
    x = np.asarray(x, dtype=np.float32)
    w_qkv = np.asarray(w_qkv, dtype=np.float32)
    b_qkv = np.asarray(b_qkv, dtype=np.float32)
    w_out = np.asarray(w_out, dtype=np.float32)
    b_out = np.asarray(b_out, dtype=np.float32)
    res = _run(_shard(x, w_qkv, b_qkv, w_out, b_out))
    return _unshard(res.results, b_qkv, w_out, b_out)


def kernel_profiled(x, w_qkv, b_qkv, w_out, b_out, tmpdir=None):
    """Like kernel() but captures an NTFF profile (requires the NTFF hook
    to be registered, e.g. via prof_shim.install()). Returns (out, result)."""
    if tmpdir is None:
        tmpdir = tempfile.mkdtemp(prefix="attn_trace_")
    x = np.asarray(x, dtype=np.float32)
    w_qkv = np.asarray(w_qkv, dtype=np.float32)
    b_qkv = np.asarray(b_qkv, dtype=np.float32)
    w_out = np.asarray(w_out, dtype=np.float32)
    b_out = np.asarray(b_out, dtype=np.float32)
    res = _run(_shard(x, w_qkv, b_qkv, w_out, b_out), trace=True,
               tmpdir=tmpdir)
    return _unshard(res.results, b_qkv, w_out, b_out), res
